# revision 1
# baseline (speedup 1.0000x reference)
"""Causal multi-head self-attention on 8 Trainium2 NeuronCores.

Problem: x[2,2048,1024], 16 heads, dk=64, causal softmax, fp32 in/out.

Sharding (data + tensor parallel, per the hint): core c handles batch
b = c//4 and head group g = c%4 (4 heads = 256 feature cols). wq/wk/wv
are column-sharded, wo row-sharded; each core returns a [D, S] partial
of out^T for its batch, and the host sums the 4 partials per batch.

Per-core kernel (layouts chosen so no on-device transposes are needed;
all matmul inputs bf16, accumulation fp32 in PSUM):
  - host supplies x^T [D, S] bf16; q^T/k^T [256, S] = w^T @ x^T on PE,
    v [S, 256] natural; v stored with a ones column per head (65-wide
    groups) so the AV matmul also produces softmax denominators.
  - scores^T tile [k=128, q<=1024] = k_h^T.T @ q_h^T, causal tiles
    only. Head pairs sit at partition bases 0/64 of the same tiles, so
    their K=64 matmuls row-tile and run concurrently on the PE array.
    The diagonal 128x128 block gets a staircase additive mask from one
    extra bf16 matmul: -240*(k-q) for k>q, 0 otherwise.
  - exp on ScalarE (scale=1/8 fused; no max-subtraction: |scores|<~3,
    masked entries underflow to exactly 0), psum -> bf16 sbuf.
  - av[65+, q] += v_aug.T @ e accumulated over k tiles (v head groups
    padded to 128 cols so weight loads take the fast path); row 64 =
    softmax denominator. vector reciprocal on row 64, broadcast across
    partitions by bouncing the row through DRAM (DMA cannot read a
    step-0 partition AP from SBUF; the gpsimd partition_broadcast and
    custom-DVE reciprocal ucodes proved broken through this runtime
    path), one tensor_mul normalizes into attnT [256, S] bf16. Odd
    heads bounce via SBUF tmp + DMA (compute engines cannot cross
    partition lanes).
  - out^T [D, S] fp32 = wo.T @ attnT on PE, evacuated on the vector
    engine, DMA'd out. Host sums the 4 partials per batch in fp64.
"""

import os
import sys

import numpy as np

if "/opt/trn_rl_repo" not in sys.path:
    sys.path.insert(0, "/opt/trn_rl_repo")

DEBUG = bool(os.environ.get("BASSDBG"))

B, S, D, H, DK = 2, 2048, 1024, 16, 64
HPC = 4            # heads per core
GW = HPC * DK      # 256
NCORES = 8
QC = 1024          # q-chunk width
NQC = S // QC      # 2
KT = 128           # k-tile
MASK_STEP = -240.0

_CACHE = {}


def _build_nc(reps=1):
    import concourse.bacc as bacc
    import concourse.tile as tile
    import concourse.bass as bass
    from concourse import mybir

    f32 = mybir.dt.float32
    bf = mybir.dt.bfloat16
    Exp = mybir.ActivationFunctionType.Exp
    PSUM = bass.MemorySpace.PSUM

    nc = bacc.Bacc(
        "TRN2",
        target_bir_lowering=False,
        debug=False,
        enable_asserts=False,
        num_devices=NCORES,
    )

    xT_d = nc.dram_tensor("xT", [D, S], bf, kind="ExternalInput")
    wq_d = nc.dram_tensor("wq", [D, GW], bf, kind="ExternalInput")
    wk_d = nc.dram_tensor("wk", [D, GW], bf, kind="ExternalInput")
    wv_d = nc.dram_tensor("wv", [D, GW], bf, kind="ExternalInput")
    wo_d = nc.dram_tensor("wo", [GW, D], bf, kind="ExternalInput")
    stA_d = nc.dram_tensor("stairA", [128, 128], bf, kind="ExternalInput")
    stB_d = nc.dram_tensor("stairB", [128, 128], bf, kind="ExternalInput")
    outT_d = nc.dram_tensor("outT", [D, S], f32, kind="ExternalOutput")
    scratch_d = nc.dram_tensor("nrm_scratch", [8, QC], f32)
    if DEBUG:
        dbg_sums_d = nc.dram_tensor("dbg_sums", [1, QC], f32, kind="ExternalOutput")
        dbg_rden_d = nc.dram_tensor("dbg_rden", [1, QC], f32, kind="ExternalOutput")
        dbg_bc_d = nc.dram_tensor("dbg_bc", [DK, QC], f32, kind="ExternalOutput")
        dbg_attnT_d = nc.dram_tensor("dbg_attnT", [128, 2, S], bf, kind="ExternalOutput")
        dbg_qT_d = nc.dram_tensor("dbg_qT", [128, 2, S], bf, kind="ExternalOutput")

    KC = D // 128  # 8 contraction chunks for the projections

    with tile.TileContext(nc) as tc:
        with (
            tc.tile_pool(name="weights", bufs=1) as wpool,
            tc.tile_pool(name="acts", bufs=1) as apool,
            tc.tile_pool(name="psmm", bufs=2, space=PSUM) as psmm,
            tc.tile_pool(name="psav", bufs=2, space=PSUM) as psav,
            tc.tile_pool(name="epool", bufs=8) as epool,
            tc.tile_pool(name="norm", bufs=3) as npool,
            tc.tile_pool(name="outp", bufs=4) as opool,
        ):
            # ---- loads ----
            # wq first, then the xT chunks: the first projection psum needs
            # wq plus all 8 xT chunks, so nothing else may delay them (the
            # stair constants are not needed until the first diagonal mask)
            stA = wpool.tile([128, 128], bf, tag="stA")
            stB = wpool.tile([128, 128], bf, tag="stB")
            wq_sb = wpool.tile([128, KC, GW], bf, tag="wq")
            wk_sb = wpool.tile([128, KC, GW], bf, tag="wk")
            wv_sb = wpool.tile([128, KC, GW], bf, tag="wv")
            wo_sb = wpool.tile([128, 2, D], bf, tag="wo")
            nc.sync.dma_start(wq_sb, wq_d.ap().rearrange("(kc p) m -> p kc m", p=128))

            first_rep = True
            for _rep in range(reps):  # >1 only for timing builds
                xT_sb = apool.tile([128, KC, S], bf, tag="xT", name=f"xT_sb{_rep}")
                xT_view = xT_d.ap().rearrange("(kc p) s -> p kc s", p=128)
                for kc in range(KC):
                    nc.sync.dma_start(xT_sb[:, kc, :], xT_view[:, kc, :])
                if first_rep:
                    first_rep = False
                    nc.sync.dma_start(
                        wk_sb, wk_d.ap().rearrange("(kc p) m -> p kc m", p=128))
                    nc.sync.dma_start(
                        wv_sb, wv_d.ap().rearrange("(kc p) m -> p kc m", p=128))
                    nc.sync.dma_start(
                        wo_sb, wo_d.ap().rearrange("(f p) n -> p f n", p=128))
                    nc.sync.dma_start(stA, stA_d.ap())
                    nc.sync.dma_start(stB, stB_d.ap())

                qT_sb = apool.tile([128, 2, S], bf, tag="qT")
                kT_sb = apool.tile([128, 2, S], bf, tag="kT")
                # head groups padded to 128 cols so AV matmul weights are
                # 128-wide (enables the compiler's fast-weight-load path);
                # cols [65,128) of each group are zeroed once on gpsimd
                v_sb = apool.tile([128, S // 128, HPC * 128], bf, tag="v")
                vpad = v_sb.rearrange("p st (h w) -> p st h w", w=128)
                nc.gpsimd.memset(vpad[:, :, :, DK + 1:128], 0.0)
                attnT = apool.tile([128, 2, S], bf, tag="attnT")

                def segs(vs):  # split [vs, QC) at the 512 psum-bank boundary
                    return [(vs, 512), (512, QC)] if vs < 512 else [(vs, QC)]

                def proj_qk(m, c2):
                    for name, w_sb, dst in (("q", wq_sb, qT_sb), ("k", wk_sb, kT_sb)):
                        ps = psmm.tile([128, QC], f32, tag="mm")
                        for kc in range(KC):
                            for a, b in segs(0):
                                nc.tensor.matmul(
                                    ps[:, a:b],
                                    lhsT=w_sb[:, kc, 128 * m:128 * (m + 1)],
                                    rhs=xT_sb[:, kc, QC * c2 + a:QC * c2 + b],
                                    start=(kc == 0),
                                    stop=(kc == KC - 1),
                                )
                        nc.vector.tensor_copy(dst[:, m, QC * c2:QC * (c2 + 1)], ps)

                def proj_v(st):
                    ps = psmm.tile([128, QC], f32, tag="mm")
                    for kc in range(KC):
                        nc.tensor.matmul(
                            ps[:, 0:GW],
                            lhsT=xT_sb[:, kc, 128 * st:128 * (st + 1)],
                            rhs=wv_sb[:, kc, :],
                            start=(kc == 0),
                            stop=(kc == KC - 1),
                        )
                    vdst = v_sb[:, st, :].rearrange("p (h w) -> p h w", w=128)
                    nc.vector.tensor_copy(
                        vdst[:, :, 0:DK],
                        ps[:, 0:GW].rearrange("p (h w) -> p h w", w=DK),
                    )
                    nc.vector.memset(vdst[:, :, DK:DK + 1], 1.0)

                def attention(mi, c):
                    # both heads of pair mi, q-chunk c; scores row-tile on PE
                    q0 = QC * c
                    njt = (q0 + QC) // KT
                    avs = []
                    for hh in range(2):
                        av = psav.tile([128, QC], f32, tag="av", name=f"av{hh}")
                        avs.append(av)
                    for j in range(njt):
                        k0 = KT * j
                        vs = max(0, k0 - q0)
                        pss = []
                        for hh in range(2):  # packed pair: bases 0 and 64
                            pb = 64 * hh
                            ps = psmm.tile([128, QC], f32, tag="mm")
                            for a, b in segs(vs):
                                diag_here = (k0 >= q0) and (a == vs)
                                nc.tensor.matmul(
                                    ps[:, a:b],
                                    lhsT=kT_sb[pb:pb + DK, mi, k0:k0 + KT],
                                    rhs=qT_sb[pb:pb + DK, mi, q0 + a:q0 + b],
                                    start=True,
                                    stop=not diag_here,
                                )
                                if diag_here:  # staircase causal mask on diag block
                                    nc.tensor.matmul(
                                        ps[:, vs:vs + KT],
                                        lhsT=stA,
                                        rhs=stB,
                                        start=False,
                                        stop=True,
                                    )
                            pss.append(ps)
                        # psum groups are tracked per 2KB bank: the first matmul
                        # touching a bank carries start, the last carries stop,
                        # partial-width writes in between are fine.
                        jA_last = q0 // KT + 3  # last j with vs < 512
                        av_ranges = []
                        if vs < 512:
                            av_ranges.append((vs, 512, j == jA_last))
                        av_ranges.append((max(vs, 512), QC, j == njt - 1))
                        for hh in range(2):
                            h = 2 * mi + hh
                            e = epool.tile([128, QC], bf, tag="e")
                            nc.scalar.activation(
                                e[:, vs:QC], pss[hh][:, vs:QC], Exp, scale=0.125
                            )
                            for a, b, fin in av_ranges:
                                nc.tensor.matmul(
                                    avs[hh][:, a:b],
                                    lhsT=v_sb[:, j, h * 128:(h + 1) * 128],
                                    rhs=e[:, a:b],
                                    start=(j == 0),
                                    stop=fin,
                                )
                    for hh in range(2):
                        av = avs[hh]
                        uid = (mi * 2 + c) * 2 + hh
                        rden = npool.tile([DK + 1, QC], f32, tag="rden")
                        nc.vector.reciprocal(rden[DK:DK + 1, :], av[DK:DK + 1, :])
                        # broadcast across partitions: bounce through DRAM (DMA
                        # cannot read a step-0 partition dim from SBUF, and
                        # compute engines cannot cross partition lanes)
                        sc = scratch_d.ap()[uid:uid + 1, :]
                        nc.sync.dma_start(sc, rden[DK:DK + 1, :])
                        bc = npool.tile([DK, QC], f32, tag="bc")
                        nc.sync.dma_start(bc, sc.to_broadcast([DK, QC]))
                        if DEBUG and mi == 1 and c == 1 and hh == 1:
                            dbg_s = npool.tile([DK + 1, QC], f32, tag="dbgs")
                            nc.vector.tensor_copy(dbg_s[DK:DK + 1, :], av[DK:DK + 1, :])
                            nc.sync.dma_start(dbg_sums_d.ap(), dbg_s[DK:DK + 1, :])
                            nc.sync.dma_start(dbg_rden_d.ap(), rden[DK:DK + 1, :])
                            nc.sync.dma_start(dbg_bc_d.ap(), bc)
                        if hh == 0:
                            nc.vector.tensor_mul(
                                attnT[0:DK, mi, q0:q0 + QC], av[0:DK, :], bc
                            )
                        else:
                            tmp = npool.tile([DK, QC], bf, tag="tmp")
                            nc.vector.tensor_mul(tmp, av[0:DK, :], bc)
                            nc.sync.dma_start(attnT[64:64 + DK, mi, q0:q0 + QC], tmp)

                def wo_proj(c2):  # output projection for one 1024-wide s-chunk
                    for dm in range(D // 128):
                        po = psmm.tile([128, QC], f32, tag="mm")
                        for f in range(2):
                            for a, b in segs(0):
                                nc.tensor.matmul(
                                    po[:, a:b],
                                    lhsT=wo_sb[:, f, 128 * dm:128 * (dm + 1)],
                                    rhs=attnT[:, f, QC * c2 + a:QC * c2 + b],
                                    start=(f == 0),
                                    stop=(f == 1),
                                )
                        ob = opool.tile([128, QC], f32, tag="ob")
                        nc.vector.tensor_copy(ob, po)
                        nc.sync.dma_start(
                            outT_d.ap()[128 * dm:128 * (dm + 1), QC * c2:QC * (c2 + 1)],
                            ob,
                        )

                # emission order: minimal prefix before attention can start;
                # later projections and the first wo chunk sit between attention
                # units so the scheduler can fill PE idle while attention waits
                # on ScalarE exp
                proj_qk(0, 0)
                proj_qk(1, 0)
                for st in range(8):
                    proj_v(st)
                attention(0, 0)
                attention(1, 0)
                proj_qk(0, 1)
                proj_qk(1, 1)
                for st in range(8, 16):
                    proj_v(st)
                attention(0, 1)
                attention(1, 1)
                wo_proj(0)
                wo_proj(1)

                if DEBUG:
                    nc.sync.dma_start(dbg_attnT_d.ap(), attnT)
                    nc.sync.dma_start(dbg_qT_d.ap(), qT_sb)

    nc.compile()
    return nc


def _get_nc():
    if "nc" not in _CACHE:
        _CACHE["nc"] = _build_nc()
    return _CACHE["nc"]


def _stairs():
    import ml_dtypes

    t = np.arange(128)
    stA = (t[:, None] <= t[None, :]).astype(ml_dtypes.bfloat16)
    stB = np.where(t[:, None] > t[None, :], MASK_STEP, 0.0).astype(ml_dtypes.bfloat16)
    return stA, stB


def _make_in_maps(x, wq, wk, wv, wo):
    import ml_dtypes

    bf = ml_dtypes.bfloat16
    stA, stB = _stairs()
    x = np.asarray(x, np.float32)
    xTs = [np.ascontiguousarray(x[b].T).astype(bf) for b in range(B)]
    wqb = np.asarray(wq, np.float32).astype(bf)
    wkb = np.asarray(wk, np.float32).astype(bf)
    wvb = np.asarray(wv, np.float32).astype(bf)
    wob = np.asarray(wo, np.float32).astype(bf)
    in_maps = []
    for c in range(NCORES):
        b, g = divmod(c, HPC)
        cols = slice(g * GW, (g + 1) * GW)
        in_maps.append({
            "xT": xTs[b],
            "wq": np.ascontiguousarray(wqb[:, cols]),
            "wk": np.ascontiguousarray(wkb[:, cols]),
            "wv": np.ascontiguousarray(wvb[:, cols]),
            "wo": np.ascontiguousarray(wob[cols, :]),
            "stairA": stA,
            "stairB": stB,
        })
    return in_maps


def run(x, wq, wk, wv, wo, trace=False):
    from concourse.bass_utils import run_bass_kernel_spmd

    nc = _get_nc()
    in_maps = _make_in_maps(x, wq, wk, wv, wo)
    res = run_bass_kernel_spmd(nc, in_maps, list(range(NCORES)), trace=trace)
    acc = np.zeros((B, D, S), np.float64)
    for c in range(NCORES):
        acc[c // HPC] += res.results[c]["outT"]
    out = np.ascontiguousarray(acc.transpose(0, 2, 1).astype(np.float32))
    return out, res


def kernel(x, wq, wk, wv, wo):
    out, _ = run(x, wq, wk, wv, wo, trace=False)
    return out

